# revision 3
# baseline (speedup 1.0000x reference)
"""InteractionMapInit Trainium2 kernel.

out[i, j, :] = tanh( tf[i] - df[j] + dnorm[i, j] )  on the block diagonal,
with tf = X@Wt + bt, df = Dft@Wd + bd.  One core per DT-pair block (B=8).

fp16 data path (inputs, tfT, output; fp32 PSUM accumulation): abs err vs
fp32 reference ~2.6e-3, 8x under the 2e-2 gate.

Per 2048-wide PSUM group (16 atoms x H, 4 banks, 2 groups in flight):
  PE : 4x mm2 (dnorm rows + [bias-df] row, K=Ap+1), then 4x mm1 (tfT @ I4)
  ACT: tanh -> fp16 SBUF
  DMA: one store per group
Lead-in tricks: r2d head + lhsT2 + split blob DMAs land before the first
matmuls; group 0's mm2s are emitted before the tf phase to warm the PE.
Host precomputes dnormT and the (bias - df) row (<3% of FLOPs); the device
does the big linear, the full map materialization, and tanh.
"""

import numpy as np

NR, NA, TD, DD, H, B = 3200, 320, 512, 128, 128, 8
NCORES = 8
P = 128

_last_results = None
_last_nc = None
_last_in_maps = None


def _host_prep(target_feature, drug_feature, target_pos, drug_pos,
               Wt, bt, Wd, bd, seg_res, seg_atom):
    f32 = np.float32
    X = np.asarray(target_feature, f32)
    Dft = np.asarray(drug_feature, f32)
    tp = np.asarray(target_pos, f32)
    dp = np.asarray(drug_pos, f32)
    Wt = np.asarray(Wt, f32)
    Wd = np.asarray(Wd, f32)
    bias = (np.asarray(bt, f32) - np.asarray(bd, f32)).reshape(1, H)
    seg_res = np.asarray(seg_res)
    seg_atom = np.asarray(seg_atom)

    r0 = np.searchsorted(seg_res, np.arange(B), side="left")
    r1 = np.searchsorted(seg_res, np.arange(B), side="right")
    a0 = np.searchsorted(seg_atom, np.arange(B), side="left")
    a1 = np.searchsorted(seg_atom, np.arange(B), side="right")
    r_cnt = (r1 - r0).astype(int)
    a_cnt = (a1 - a0).astype(int)

    Rp = max(P, int(-(-max(r_cnt) // P)) * P)
    Ap = max(4, int(-(-max(a_cnt) // 4)) * 4)
    assert Ap + 1 <= 128, f"block atom count too large: {max(a_cnt)}"
    AH = Ap * H
    K_TD = TD // P

    # blob columns: wt[k-major] | xt[k-major]
    CB = K_TD * H + K_TD * Rp
    R2D_delta = np.kron(np.eye(Ap, dtype=f32), np.ones((1, H), f32))
    wt_km = np.ascontiguousarray(
        Wt.reshape(K_TD, P, H).transpose(1, 0, 2).reshape(P, K_TD * H))

    f16 = np.float16
    in_maps = []
    for c in range(B):
        rc, ac = r_cnt[c], a_cnt[c]
        blob = np.zeros((P, CB), f32)
        xt = np.zeros((TD, Rp), f32)
        if rc > 0:
            xt[:, :rc] = X[r0[c]:r1[c]].T
        blob[:, :K_TD * H] = wt_km
        # xt: per-partition layout (rt, k, i128) so each row tile's features
        # are one contiguous [P, K_TD*P] DMA
        blob[:, K_TD * H:] = (
            xt.reshape(K_TD, P, Rp // P, P).transpose(1, 2, 0, 3)
              .reshape(P, K_TD * Rp))

        r2d = np.zeros((Ap + 1, AH), f32)
        r2d[:Ap] = R2D_delta
        dfrow = np.tile(bias, (Ap, 1))
        if ac > 0:
            dfrow[:ac] -= Dft[a0[c]:a1[c]] @ Wd
        r2d[Ap] = dfrow.ravel()

        lhsT2 = np.zeros((Ap + 1, Rp), f32)
        lhsT2[Ap, :] = 1.0
        if rc > 0 and ac > 0:
            d = tp[r0[c]:r1[c], None, :] - dp[None, a0[c]:a1[c], :]
            D = np.sqrt((d * d).sum(-1, dtype=f32), dtype=f32)
            dmin, dmax = float(D.min()), float(D.max())
            denom = (dmax - dmin) if dmax > dmin else 1.0
            lhsT2[:ac, :rc] = ((D - dmin) / denom).T

        in_maps.append({
            "blob": np.ascontiguousarray(blob.astype(f16)),
            "lhst2": np.ascontiguousarray(lhsT2.astype(f16)),
            "r2d": np.ascontiguousarray(r2d.astype(f16)),
        })

    meta = dict(r0=r0, a0=a0, r_cnt=r_cnt, a_cnt=a_cnt, Rp=Rp, Ap=Ap)
    return in_maps, meta


def build_bass(Rp, Ap, grp=3, big_bufs=2):
    from contextlib import ExitStack

    import concourse.bacc as bacc
    import concourse.mybir as mybir
    import concourse.tile as tile

    F32 = mybir.dt.float32
    F16 = mybir.dt.float16
    AF = mybir.ActivationFunctionType
    OP = mybir.AluOpType

    K_TD = TD // P        # 4 contraction chunks for the target linear
    RT = Rp // P          # 128-row tiles
    NCH = Ap // 4         # 512-wide psum chunks (4 atoms x H)
    AH = Ap * H
    CB = K_TD * H + K_TD * Rp

    nc = bacc.Bacc("TRN2", target_bir_lowering=False, debug=False,
                   num_devices=NCORES)

    blob_d = nc.dram_tensor("blob", [P, CB], F16, kind="ExternalInput").ap()
    lhst2_d = nc.dram_tensor("lhst2", [Ap + 1, Rp], F16, kind="ExternalInput").ap()
    r2d_d = nc.dram_tensor("r2d", [Ap + 1, AH], F16, kind="ExternalInput").ap()
    out_d = nc.dram_tensor("out", [Rp, AH], F16, kind="ExternalOutput").ap()

    with tile.TileContext(nc) as tc, ExitStack() as ctx:
        singles = ctx.enter_context(tc.tile_pool(name="singles", bufs=1))
        psumb = ctx.enter_context(
            tc.tile_pool(name="psumb", bufs=big_bufs, space="PSUM"))
        psumtf = ctx.enter_context(
            tc.tile_pool(name="psumtf", bufs=2, space="PSUM"))
        outs = ctx.enter_context(tc.tile_pool(name="outs", bufs=4))

        # ------------- input DMAs, earliest-needed first -------------
        gw0 = 512 * grp
        RT_ = Rp // P
        blob = singles.tile([P, CB], F16, name="blob")
        r2d_sb = singles.tile([Ap + 1, AH], F16, name="r2d_sb")
        lhsT2 = singles.tile([Ap + 1, Rp], F16, name="lhsT2")

        def xt_dma(rt):
            o = K_TD * H + rt * (K_TD * P)
            nc.sync.dma_start(out=blob[:, o:o + K_TD * P],
                              in_=blob_d[:, o:o + K_TD * P])

        # wt + first row tile's features in one transfer; r2d split per
        # chunk-group, interleaved so each lands just before it is needed
        wx0 = K_TD * H + K_TD * P
        nc.sync.dma_start(out=blob[:, :wx0], in_=blob_d[:, :wx0])
        nc.sync.dma_start(out=r2d_sb[:, :gw0], in_=r2d_d[:, :gw0])
        nc.sync.dma_start(out=lhsT2, in_=lhst2_d)
        nc.sync.dma_start(out=r2d_sb[:, gw0:2 * gw0], in_=r2d_d[:, gw0:2 * gw0])
        nc.sync.dma_start(out=blob[:, wx0:], in_=blob_d[:, wx0:])
        pos = 2 * gw0
        while pos < AH:
            e = min(pos + gw0, AH)
            nc.sync.dma_start(out=r2d_sb[:, pos:e], in_=r2d_d[:, pos:e])
            pos = e
        wt_sb = blob[:, :K_TD * H].rearrange("p (k h) -> p k h", k=K_TD)
        xt_sb = blob[:, K_TD * H:].rearrange("p (t k i) -> p t k i",
                                             t=RT_, k=K_TD)

        # -------- PE warm-up: dependency-free dummy matmuls so the PE
        # -------- clock-ramp (full speed only after ~3us of continuous
        # -------- busy) completes before the first real matmuls. Reads
        # -------- uninitialized SBUF, writes a dead PSUM tile.
        junk = singles.tile([P, P], F32, name="junk")
        nc.gpsimd.memset(junk, 1.0)
        dead = psumtf.tile([P, P], F32, tag="ps_tf", name="dead")
        for _ in range(6):
            nc.tensor.matmul(dead, lhsT=junk, rhs=junk, start=True, stop=True)

        # ---------------- I4 = [I I I I] (device-built) ----------------
        ones_sb = singles.tile([P, P], F32, name="ones_sb")
        nc.vector.memset(ones_sb, 1.0)
        i4_sb = singles.tile([P, 512], F16, name="i4_sb")
        for k in range(4):
            nc.gpsimd.affine_select(
                out=i4_sb[:, P * k:P * (k + 1)], in_=ones_sb,
                pattern=[[1, P]], compare_op=OP.is_equal, fill=0.0,
                base=0, channel_multiplier=-1)

        # -------- main loop; tf for each row tile computed just-in-time ----
        tfT = singles.tile([P, Rp], F16, name="tfT")
        for rt in range(RT):
            rsl = slice(P * rt, P * (rt + 1))
            ps_tf = psumtf.tile([P, P], F32, tag="ps_tf", name="ps_tf")
            for k in range(K_TD):
                nc.tensor.matmul(ps_tf[:, :P], lhsT=wt_sb[:, k, :],
                                 rhs=xt_sb[:, rt, k, :],
                                 start=(k == 0), stop=(k == K_TD - 1))
            nc.vector.tensor_copy(out=tfT[:, rsl], in_=ps_tf[:, :P])
            pos = 0
            while pos < NCH:
                g = min(grp, NCH - pos)
                gw = g * 512
                pso = psumb.tile([P, grp * 512], F32, tag="ps_big", name="pso")
                for c in range(g):
                    ch = pos + c
                    nc.tensor.matmul(
                        pso[:, 512 * c:512 * (c + 1)], lhsT=lhsT2[:, rsl],
                        rhs=r2d_sb[:, 512 * ch:512 * (ch + 1)],
                        start=True, stop=False)
                for c in range(g):
                    nc.tensor.matmul(pso[:, 512 * c:512 * (c + 1)],
                                     lhsT=tfT[:, rsl], rhs=i4_sb,
                                     start=False, stop=True)
                ob = outs.tile([P, grp * 512], F16, name="ob")
                last = (rt == RT - 1 and pos + g >= NCH)
                if last and g >= 2:
                    # split the final tanh+store so the last DMA is half-size
                    h1 = (g - 1) * 512
                    nc.scalar.activation(out=ob[:, :h1], in_=pso[:, :h1],
                                         func=AF.Tanh)
                    nc.sync.dma_start(
                        out=out_d[rsl, 512 * pos:512 * pos + h1],
                        in_=ob[:, :h1])
                    nc.scalar.activation(out=ob[:, h1:gw], in_=pso[:, h1:gw],
                                         func=AF.Tanh)
                    nc.sync.dma_start(
                        out=out_d[rsl, 512 * pos + h1:512 * pos + gw],
                        in_=ob[:, h1:gw])
                else:
                    nc.scalar.activation(out=ob[:, :gw], in_=pso[:, :gw],
                                         func=AF.Tanh)
                    nc.sync.dma_start(out=out_d[rsl, 512 * pos:512 * pos + gw],
                                      in_=ob[:, :gw])
                pos += g

    nc.compile()
    return nc


def kernel(**inputs) -> np.ndarray:
    global _last_results, _last_nc, _last_in_maps
    in_maps, meta = _host_prep(**inputs)
    Rp, Ap = meta["Rp"], meta["Ap"]

    nc = build_bass(Rp, Ap)
    _last_nc, _last_in_maps = nc, in_maps

    from concourse.bass_utils import run_bass_kernel_spmd
    res = run_bass_kernel_spmd(nc, in_maps, core_ids=list(range(NCORES)))
    _last_results = res

    out = np.zeros((NR, NA, H), np.float32)
    for c in range(B):
        rc, ac = int(meta["r_cnt"][c]), int(meta["a_cnt"][c])
        if rc == 0 or ac == 0:
            continue
        blk = res.results[c]["out"].astype(np.float32).reshape(Rp, Ap, H)
        r0, a0 = int(meta["r0"][c]), int(meta["a0"][c])
        out[r0:r0 + rc, a0:a0 + ac, :] = blk[:rc, :ac, :]
    return out
